# revision 1
# baseline (speedup 1.0000x reference)
"""AlibiCausalSelfAttention on 8 Trainium2 NeuronCores — v2.

Sharding: data-parallel over batch (B=2) x head-parallel over head groups
(16 heads -> 4 groups of 4). Core c handles batch c//4, heads [4*(c%4), 4*(c%4)+4).
Each core computes a partial projection output [T, C] fp16 (W_proj row-sharded);
the host sums the 4 partials per batch in fp32 and adds b_proj.

v2 structural changes vs v1 (221us):
  - qk bias folded into the projection matmul as a 9th K=1 accumulation step
    (lhsT = bias row, rhs = ones row); PSUM->SBUF evacuations become pure
    cast-copies: qk on DVE, v on ACT (idle during phase 1).
  - v tiles store [64 v-dims | 64 ones] per head, so the PV matmul (M=128)
    broadcasts the softmax denominator into PSUM rows 64..127 for free.
    Normalization is then just reciprocal_approx_fast (rows 64:128 -> 0:64
    partition shift) + one tensor_mul. Replaces v1's reciprocal/broadcast-
    matmul/copy chain (~58us DVE -> ~22us).
  - St pair matmuls placed at packed column offsets [0:Wa|Wa:Wa+Wb] so exp
    covers one contiguous trimmed range; exp writes fp16 directly.  Diagonal
    causal masking via a single tensor_tensor MIN against a {30000,0} mask
    (also squashes fp16 exp overflow inf -> 0).
  - Software-pipelined St(n+1) before PV(n) so the in-order PE queue never
    stalls on ACT's exp.
  - Single tile scope; program order interleaves phase 1 (heads 0,1 first),
    attention, and output projection so all engines stay busy and the PE
    HAM clock gate stays warm.
  - fp16 output (halves the store DMA).
"""

import sys

if "/opt/trn_rl_repo" not in sys.path:
    sys.path.insert(0, "/opt/trn_rl_repo")

import numpy as np

T = 2048
C = 1024
H = 16
D = 64
HL = 4          # heads per core
HD = HL * D     # 256 local head dims
IW = 512        # i-tile width

_CACHE = {}


def _build_nc():
    import concourse.mybir as mybir
    import concourse.tile as tile
    from concourse import bacc
    from contextlib import ExitStack

    f32 = mybir.dt.float32
    fr = mybir.dt.float16
    Exp = mybir.ActivationFunctionType.Exp
    Min = mybir.AluOpType.min

    nc = bacc.Bacc("TRN2", target_bir_lowering=False, debug=False, num_devices=8)

    xT = nc.dram_tensor("xT", [C, T], fr, kind="ExternalInput").ap()
    wqk = nc.dram_tensor("wqk", [C, 2 * HD], fr, kind="ExternalInput").ap()
    bqk = nc.dram_tensor("bqk", [1, 2 * HD], fr, kind="ExternalInput").ap()
    wv = nc.dram_tensor("wv", [C, HD], fr, kind="ExternalInput").ap()
    bv = nc.dram_tensor("bv", [1, HD], fr, kind="ExternalInput").ap()
    wp = nc.dram_tensor("wp", [HD, C], fr, kind="ExternalInput").ap()
    aq = nc.dram_tensor("aq", [2 * HL, T], fr, kind="ExternalInput").ap()
    ak = nc.dram_tensor("ak", [2, T], fr, kind="ExternalInput").ap()
    umin_d = nc.dram_tensor("umin", [128, 128], fr, kind="ExternalInput").ap()
    onesr_d = nc.dram_tensor("onesr", [1, IW], fr, kind="ExternalInput").ap()
    out = nc.dram_tensor("out", [T, C], fr, kind="ExternalOutput").ap()

    NT16 = T // 128  # 16 t-chunks

    with tile.TileContext(nc) as tc, ExitStack() as ctx:
        pers = ctx.enter_context(tc.tile_pool(name="pers", bufs=1))
        work = ctx.enter_context(tc.tile_pool(name="work", bufs=3))
        rpool = ctx.enter_context(tc.tile_pool(name="rpool", bufs=2))
        ps_st = ctx.enter_context(tc.tile_pool(name="ps_st", bufs=2, space="PSUM"))
        ps_y = ctx.enter_context(tc.tile_pool(name="ps_y", bufs=2, space="PSUM"))
        ps_o = ctx.enter_context(tc.tile_pool(name="ps_o", bufs=2, space="PSUM"))

        # ---- persistent tiles ----
        qaug = [pers.tile([128, T], fr, tag=f"qaug{h}", name=f"qaug{h}") for h in range(HL)]
        kaug = [pers.tile([128, T], fr, tag=f"kaug{h}", name=f"kaug{h}") for h in range(HL)]
        # v tiles: per t-chunk [128, 4 heads, 128]; cols 0:64 = v data, 64:128 = ones
        vaug = [pers.tile([128, HL, 128], fr, tag=f"vaug{t}", name=f"vaug{t}")
                for t in range(NT16)]
        yT = [pers.tile([128, T], fr, tag=f"yT{i}", name=f"yT{i}") for i in range(HL // 2)]
        # consolidated weight/input tiles: C-chunk k is a middle free dim so a
        # single DMA (rearranged DRAM AP) loads all chunks
        xs_t = pers.tile([128, 8, T], fr, tag="xs")
        wqk_t = pers.tile([128, 8, 2 * HD], fr, tag="wqkt")
        wv_t = pers.tile([128, 8, HD], fr, tag="wvt")
        wp_t = pers.tile([128, 2, C], fr, tag="wpt")
        xs = [xs_t[:, k, :] for k in range(8)]
        wqks = [wqk_t[:, k, :] for k in range(8)]
        wvs = [wv_t[:, k, :] for k in range(8)]
        wp_sb = [wp_t[:, i, :] for i in range(2)]
        bqk_sb = pers.tile([1, 2 * HD], fr, tag="bqk")
        bv_sb = pers.tile([1, HD], fr, tag="bv")
        umin = pers.tile([128, 128], fr, tag="umin")
        onesrow = pers.tile([1, IW], fr, tag="onesrow")

        # ---- DMAs ordered by first consumption; consolidated via rearranged
        # DRAM APs (chunk dim k folded into the free dims).  x streams in
        # 512-column quarters so early qkT tiles can start immediately.
        nc.sync.dma_start(bqk_sb[:], bqk[:])
        nc.sync.dma_start(bv_sb[:], bv[:])
        nc.sync.dma_start(onesrow[:], onesr_d[:])
        wqk_r = wqk.rearrange("(k p) c -> p k c", p=128)
        xT_r = xT.rearrange("(k p) t -> p k t", p=128)
        nc.sync.dma_start(wqk_t[:, 0:2, :], wqk_r[:, 0:2, :])
        # tiny constant loads issue from the (idle) ACT HWDGE queue so they
        # don't consume sync-queue issue slots ahead of the x stream
        nc.scalar.dma_start(umin[:], umin_d[:])
        for h in range(HL):
            nc.scalar.dma_start(qaug[h][64:66, :], aq[2 * h:2 * h + 2, :])
            nc.scalar.dma_start(kaug[h][64:66, :], ak[:, :])
        nc.sync.dma_start(xs_t[:, 0:2, 0:IW], xT_r[:, 0:2, 0:IW])
        nc.sync.dma_start(wqk_t[:, 2:8, :], wqk_r[:, 2:8, :])
        nc.sync.dma_start(xs_t[:, 2:8, 0:IW], xT_r[:, 2:8, 0:IW])
        for tt in range(1, 4):
            nc.sync.dma_start(
                xs_t[:, :, tt * IW:(tt + 1) * IW],
                xT_r[:, :, tt * IW:(tt + 1) * IW])
        nc.sync.dma_start(wv_t[:], wv.rearrange("(k p) c -> p k c", p=128))
        # ones columns of vaug (everything 1.0; v-evac overwrites cols 0:64)
        for t in range(NT16):
            nc.vector.memset(vaug[t][:], 1.0)
        nc.sync.dma_start(wp_t[:], wp.rearrange("(i p) c -> p i c", p=128))

        # ---------------- phase 1 helpers ----------------
        def qk_tile(cc, tt):
            # psum [128, IW] = wqk cols [cc*128:(cc+1)*128] x tokens tt*IW..
            # (shares the outproj pool: phase-1 and outproj never overlap)
            ps = ps_o.tile([128, IW], f32, tag="o", name="qkps")
            for k in range(8):
                nc.tensor.matmul(
                    ps[:], wqks[k][:, cc * 128:(cc + 1) * 128],
                    xs[k][:, tt * IW:(tt + 1) * IW],
                    start=(k == 0), stop=False)
            nc.tensor.matmul(
                ps[:], bqk_sb[0:1, cc * 128:(cc + 1) * 128], onesrow[:],
                start=False, stop=True)
            for half in range(2):
                h = (cc % 2) * 2 + half
                dst = qaug[h] if cc < 2 else kaug[h]
                nc.vector.tensor_copy(
                    dst[0:64, tt * IW:(tt + 1) * IW],
                    ps[half * 64:(half + 1) * 64, :])

        def v_tile(t16):
            ps = ps_o.tile([128, HD], f32, tag="o", name="vps")
            for k in range(8):
                nc.tensor.matmul(
                    ps[:], xs[k][:, t16 * 128:(t16 + 1) * 128], wvs[k][:],
                    start=(k == 0), stop=False)
            nc.tensor.matmul(
                ps[:], onesrow[:, 0:128], bv_sb[:], start=False, stop=True)
            for h in range(HL):
                nc.vector.tensor_copy(
                    vaug[t16][:, h, 0:64], ps[:, h * 64:(h + 1) * 64])

        # ---------------- attention for one (i-tile, head) ----------------
        def attn(it, h):
            i0 = it * IW
            njc = i0 // 128 + IW // 128
            npair = njc // 2
            yacc = ps_y.tile([128, IW], f32, tag="yacc", name="yacc")
            pend = None  # (p tile, widths) of the pair whose PV is not yet emitted

            def emit_pv(p, c0a, c0b, Wa, Wb, pj):
                nc.tensor.matmul(
                    yacc[:, c0a:IW], vaug[2 * pj][:, h, :], p[:, 0:Wa],
                    start=(pj == 0), stop=False)
                nc.tensor.matmul(
                    yacc[:, c0b:IW], vaug[2 * pj + 1][:, h, :], p[:, Wa:Wa + Wb],
                    start=False, stop=(pj == npair - 1))

            for pj in range(npair):
                j0a = (2 * pj) * 128
                j0b = j0a + 128
                c0a = max(0, j0a - i0)
                c0b = max(0, j0b - i0)
                Wa = IW - c0a
                Wb = IW - c0b
                st2 = ps_st.tile([128, 2 * IW], f32, tag="st", name="st")
                nc.tensor.matmul(
                    st2[:, 0:Wa],
                    kaug[h][0:66, j0a:j0a + 128],
                    qaug[h][0:66, i0 + c0a:i0 + IW],
                    start=True, stop=True)
                nc.tensor.matmul(
                    st2[:, Wa:Wa + Wb],
                    kaug[h][0:66, j0b:j0b + 128],
                    qaug[h][0:66, i0 + c0b:i0 + IW],
                    start=True, stop=True)
                if pend is not None:
                    emit_pv(*pend)
                    pend = None
                p = work.tile([128, 2 * IW], fr, tag="p", name="p")
                nc.scalar.activation(p[:, 0:Wa + Wb], st2[:, 0:Wa + Wb], Exp)
                if j0a >= i0:
                    # diagonal squares at p cols [0:128] (chunk a) and
                    # [Wa:Wa+128] (chunk b): min with {30000 keep, 0 drop}
                    nc.vector.tensor_tensor(p[:, 0:128], p[:, 0:128], umin[:], Min)
                    nc.vector.tensor_tensor(
                        p[:, Wa:Wa + 128], p[:, Wa:Wa + 128], umin[:], Min)
                pend = (p, c0a, c0b, Wa, Wb, pj)
            emit_pv(*pend)
            # normalization: rows 64:128 of yacc hold the denominator
            # (broadcast by the 64 ones-columns of vaug).
            den = rpool.tile([64, IW], f32, tag="den", name="den")
            nc.vector.tensor_copy(den[:], yacc[64:128, :])
            rec = rpool.tile([64, IW], f32, tag="rec", name="rec")
            nc.vector.reciprocal_approx_fast(rec[:], den[:])
            nc.vector.tensor_mul(
                yT[h // 2][(h % 2) * 64:(h % 2) * 64 + 64, i0:i0 + IW],
                yacc[0:64, :], rec[:])

        def outproj(it):
            for tp in range(2 * it, 2 * it + 2):  # pairs of t-chunks
                ot = work.tile([128, 2, C], fr, tag="ot", name="ot")
                for half in range(2):
                    t16 = 2 * tp + half
                    for e2 in range(2):
                        ps = ps_o.tile([128, 512], f32, tag="o", name="ops")
                        for kk in range(2):
                            nc.tensor.matmul(
                                ps[:],
                                yT[kk][:, t16 * 128:(t16 + 1) * 128],
                                wp_sb[kk][:, e2 * 512:(e2 + 1) * 512],
                                start=(kk == 0), stop=(kk == 1))
                        nc.vector.tensor_copy(
                            ot[:, half, e2 * 512:(e2 + 1) * 512], ps[:])
                out_r = out[tp * 256:(tp + 1) * 256, :].rearrange(
                    "(a p) c -> p a c", p=128)
                nc.sync.dma_start(out_r, ot[:])

        # ---------------- program order ----------------
        # HAM warmup: junk matmuls spanning the initial DMA wait keep the PE
        # clock gate at 8/8 so real matmuls run at 2.4 GHz from the start.
        wdum = pers.tile([128, IW], fr, tag="wdum")
        nc.vector.memset(wdum[:], 0.0)
        for w in range(32):
            psd = ps_st.tile([128, IW], f32, tag="st", name="psd")
            nc.tensor.matmul(psd[:], wdum[:, 0:128], wdum[:], start=True, stop=True)
        # Fine-grained interleave: each attention block is preceded only by
        # the projection tiles it needs; qk/v/outproj fill PE slack during
        # ACT-bound attention stretches without touching the St psum slots.
        for it in range(4):
            qk_tile(0, it)
            qk_tile(2, it)
            for t16 in range(4 * it, 4 * it + 4):
                v_tile(t16)
            if it > 0:
                outproj(it - 1)
            attn(it, 0)
            attn(it, 1)
            qk_tile(1, it)
            qk_tile(3, it)
            attn(it, 2)
            attn(it, 3)
        outproj(3)

    nc.compile()
    return nc


def _get_nc():
    if "nc" not in _CACHE:
        _CACHE["nc"] = _build_nc()
    return _CACHE["nc"]


def _shard_inputs(x, W_attn, b_attn, W_proj, b_proj):
    f16 = np.float16
    slopes = (1.0 / np.power(2.0, np.arange(1, H + 1))).astype(np.float32)
    iota = np.arange(T, dtype=np.float32)
    ak = np.stack([np.ones(T, np.float32), iota]).astype(f16)      # [2, T]
    # min-mask for diagonal squares: keep (j<=i) -> 30000, drop -> 0
    pp, ff = np.meshgrid(np.arange(128), np.arange(128), indexing="ij")
    umin = np.where(pp <= ff, 30000.0, 0.0).astype(f16)
    xTs = [np.ascontiguousarray(x[b].T).astype(f16) for b in range(x.shape[0])]

    in_maps = []
    for core in range(8):
        b, g = core // 4, core % 4
        cs = slice(g * HD, (g + 1) * HD)
        q_cols = W_attn[:, 0:C][:, cs] * 0.125
        k_cols = W_attn[:, C:2 * C][:, cs]
        v_cols = np.ascontiguousarray(W_attn[:, 2 * C:3 * C][:, cs])
        wqk_l = np.ascontiguousarray(np.concatenate([q_cols, k_cols], axis=1))
        bqk_l = np.concatenate(
            [b_attn[0:C][cs] * 0.125, b_attn[C:2 * C][cs]])[None, :]
        bv_l = b_attn[2 * C:3 * C][cs][None, :]
        wp_l = np.ascontiguousarray(W_proj[g * HD:(g + 1) * HD, :])
        aq = np.zeros((2 * HL, T), np.float32)
        for hh in range(HL):
            s = slopes[g * HL + hh]
            aq[2 * hh, :] = -s * iota
            aq[2 * hh + 1, :] = s
        in_maps.append({
            "xT": xTs[b], "wqk": wqk_l.astype(f16),
            "bqk": np.ascontiguousarray(bqk_l).astype(f16),
            "wv": v_cols.astype(f16), "bv": np.ascontiguousarray(bv_l).astype(f16),
            "wp": wp_l.astype(f16), "aq": aq.astype(f16), "ak": ak,
            "umin": umin,
            "onesr": np.ones((1, IW), f16),
        })
    return in_maps


def kernel(x, W_attn, b_attn, W_proj, b_proj, _trace=False, _tmpdir=None):
    from concourse.bass_utils import run_bass_kernel_spmd

    x = np.asarray(x, dtype=np.float32)
    W_attn = np.asarray(W_attn, dtype=np.float32)
    b_attn = np.asarray(b_attn, dtype=np.float32)
    W_proj = np.asarray(W_proj, dtype=np.float32)
    b_proj = np.asarray(b_proj, dtype=np.float32)

    nc = _get_nc()
    in_maps = _shard_inputs(x, W_attn, b_attn, W_proj, b_proj)
    res = run_bass_kernel_spmd(
        nc, in_maps, core_ids=list(range(8)), trace=_trace, tmpdir=_tmpdir)
    out = np.empty((x.shape[0], T, C), np.float32)
    for b in range(x.shape[0]):
        acc = res.results[4 * b]["out"].astype(np.float32)
        for i in range(1, 4):
            acc += res.results[4 * b + i]["out"].astype(np.float32)
        out[b] = acc + b_proj
    if _trace:
        kernel.last_exec_time_ns = res.exec_time_ns
    return out



# revision 6
# speedup vs baseline: 1.1786x; 1.1786x over previous
"""AlibiCausalSelfAttention on 8 Trainium2 NeuronCores — v3.

Sharding: data-parallel over batch (B=2) x head-parallel over head groups
(16 heads -> 4 groups of 4, strided: group g = {g, g+4, g+8, g+12}).
Core c handles batch c//4, head group c%4. Each core computes a partial
projection output fp16 (W_proj row-sharded); the host sums the 4 partials
per batch in fp32 and adds an effective bias (b_proj + bv @ W_proj — the
v-bias commutes through softmax exactly).

v3 structural changes vs v2 (175us):
  - All bulk DMAs use host-prepacked per-partition-contiguous layouts
    (~128 descriptors/MB instead of ~1000), cutting DGE trigger time and
    letting the input stream start right after the ~7us framework preamble.
  - Bias matmuls eliminated: v-bias folded into the host-side output bias
    (exact); k-bias dropped (q.bk + bq.bk are per-row constants — exact
    softmax invariance); q-bias applied during the PSUM->SBUF evacuation
    via tensor_scalar_add (free).
  - Warmup junk matmuls read an uninitialized SBUF tile (no memset dep),
    so they issue as soon as the PE queue preamble ends and keep the HAM
    clock gate warm through the input-DMA window.
  - Banded attention for the steepest head of each group (global heads
    0-3, local slot 0): only j-pairs within 512 tokens of the diagonal are
    computed (dropped softmax terms < e^-18 relative — far below fp16
    resolution). Identical program on all cores; ~9% less QK/PV/exp work.
  - v-evacuation merged to one strided copy per t-chunk; softmax
    normalization reads the denominator rows straight out of PSUM
    (reciprocal_approx_fast), dropping the staging copy.
  - vaug ones-columns memset on the idle GPSIMD engine.
"""

import sys

if "/opt/trn_rl_repo" not in sys.path:
    sys.path.insert(0, "/opt/trn_rl_repo")

import numpy as np

T = 2048
C = 1024
H = 16
D = 64
HL = 4          # heads per core
HD = HL * D     # 256 local head dims
IW = 512        # i-tile width
NT16 = T // 128
NW = 12         # warmup junk matmuls

# local-slot bands (tokens below diagonal, multiple of 256; None = full).
# Slot 0 carries global heads 0..3 (slopes 1/2..1/16): need >= 18/slope =
# 288 tokens; 512 is safely generous. Slots 1-3 carry heads 4-15 (full).
BANDS = [512, None, None, None]

_CACHE = {}


def _build_nc():
    import concourse.mybir as mybir
    import concourse.tile as tile
    from concourse import bacc
    from contextlib import ExitStack

    f32 = mybir.dt.float32
    fr = mybir.dt.float16
    Exp = mybir.ActivationFunctionType.Exp
    Min = mybir.AluOpType.min

    def pj0(it, h):
        b = BANDS[h]
        if b is None:
            return 0
        return max(0, (it * IW - b) // 256)

    nc = bacc.Bacc("TRN2", target_bir_lowering=False, debug=False, num_devices=8)

    xq_d = nc.dram_tensor("xq", [128, 4, 8, IW], fr, kind="ExternalInput").ap()
    wqk_d = nc.dram_tensor("wqk", [128, 4, 8, 128], fr, kind="ExternalInput").ap()
    bq_d = nc.dram_tensor("bq", [128, 2], f32, kind="ExternalInput").ap()
    wv_d = nc.dram_tensor("wv", [128, 8, HD], fr, kind="ExternalInput").ap()
    wp_d = nc.dram_tensor("wp", [128, 2, C], fr, kind="ExternalInput").ap()
    aq_d = nc.dram_tensor("aq", [2 * HL, T], fr, kind="ExternalInput").ap()
    ak_d = nc.dram_tensor("ak", [2, T], fr, kind="ExternalInput").ap()
    umin_d = nc.dram_tensor("umin", [128, 128], fr, kind="ExternalInput").ap()
    out_d = nc.dram_tensor("out", [128, NT16, C], fr, kind="ExternalOutput").ap()

    with tile.TileContext(nc) as tc, ExitStack() as ctx:
        pers = ctx.enter_context(tc.tile_pool(name="pers", bufs=1))
        work = ctx.enter_context(tc.tile_pool(name="work", bufs=3))
        rpool = ctx.enter_context(tc.tile_pool(name="rpool", bufs=2))
        ps_st = ctx.enter_context(tc.tile_pool(name="ps_st", bufs=2, space="PSUM"))
        ps_y = ctx.enter_context(tc.tile_pool(name="ps_y", bufs=2, space="PSUM"))
        ps_o = ctx.enter_context(tc.tile_pool(name="ps_o", bufs=2, space="PSUM"))

        # ---- persistent tiles ----
        qaug = [pers.tile([128, T], fr, tag=f"qaug{h}", name=f"qaug{h}") for h in range(HL)]
        kaug = [pers.tile([128, T], fr, tag=f"kaug{h}", name=f"kaug{h}") for h in range(HL)]
        vaug = [pers.tile([128, HL, 128], fr, tag=f"vaug{t}", name=f"vaug{t}")
                for t in range(NT16)]
        yT = [pers.tile([128, T], fr, tag=f"yT{i}", name=f"yT{i}") for i in range(HL // 2)]
        xs_t = pers.tile([128, 4, 8, IW], fr, tag="xs")
        wqk_t = pers.tile([128, 4, 8, 128], fr, tag="wqkt")
        wv_t = pers.tile([128, 8, HD], fr, tag="wvt")
        wp_t = pers.tile([128, 2, C], fr, tag="wpt")
        bq_t = pers.tile([128, 2], f32, tag="bq")
        umin = pers.tile([128, 128], fr, tag="umin")
        junk = pers.tile([128, IW], fr, tag="junk")

        # ---- warmup junk matmuls: minimal deps (one gpsimd memset), keep the
        # PE/HAM warm while inputs stream in.
        nc.gpsimd.memset(junk[:], 0.0)
        for w in range(NW):
            psd = ps_st.tile([128, IW], f32, tag="st", name="psd")
            nc.tensor.matmul(psd[:], junk[:, 0:128], junk[:], start=True, stop=True)

        # ---- DMAs: per-partition-contiguous both sides; ordered so the
        # minimal prefix for early compute lands first.
        nc.sync.dma_start(wqk_t[:, 0, :, :], wqk_d[:, 0, :, :])
        nc.sync.dma_start(xs_t[:, 0, :, :], xq_d[:, 0, :, :])
        nc.sync.dma_start(wqk_t[:, 2, :, :], wqk_d[:, 2, :, :])
        nc.sync.dma_start(wv_t[:], wv_d[:])
        nc.sync.dma_start(xs_t[:, 1, :, :], xq_d[:, 1, :, :])
        nc.sync.dma_start(wqk_t[:, 1, :, :], wqk_d[:, 1, :, :])
        nc.sync.dma_start(wqk_t[:, 3, :, :], wqk_d[:, 3, :, :])
        nc.sync.dma_start(wp_t[:], wp_d[:])
        nc.sync.dma_start(xs_t[:, 2, :, :], xq_d[:, 2, :, :])
        nc.sync.dma_start(xs_t[:, 3, :, :], xq_d[:, 3, :, :])
        # small constants on the scalar HWDGE queue
        nc.scalar.dma_start(bq_t[:], bq_d[:])
        nc.scalar.dma_start(umin[:], umin_d[:])
        for h in range(HL):
            nc.scalar.dma_start(qaug[h][64:66, :], aq_d[2 * h:2 * h + 2, :])
            nc.scalar.dma_start(kaug[h][64:66, :], ak_d[:, :])
        # ones columns of vaug on the idle GPSIMD engine
        for t in range(NT16):
            nc.gpsimd.memset(vaug[t][:, :, 64:128], 1.0)

        # ---------------- phase 1 helpers ----------------
        def qk_tile(cc, it):
            ps = ps_o.tile([128, IW], f32, tag="o", name="qkps")
            for k in range(8):
                nc.tensor.matmul(
                    ps[:], wqk_t[:, cc, k, :], xs_t[:, it, k, :],
                    start=(k == 0), stop=(k == 7))
            for half in range(2):
                h = (cc % 2) * 2 + half
                src = ps[half * 64:(half + 1) * 64, :]
                if cc < 2:
                    nc.vector.tensor_scalar_add(
                        qaug[h][0:64, it * IW:(it + 1) * IW], src,
                        bq_t[half * 64:(half + 1) * 64, cc:cc + 1])
                else:
                    nc.vector.tensor_copy(
                        kaug[h][0:64, it * IW:(it + 1) * IW], src)

        def v_tile(t16):
            ps = ps_o.tile([128, HL, 64], f32, tag="o", name="vps")
            qq, sub = t16 // 4, (t16 % 4) * 128
            for k in range(8):
                nc.tensor.matmul(
                    ps[:], xs_t[:, qq, k, sub:sub + 128], wv_t[:, k, :],
                    start=(k == 0), stop=(k == 7))
            nc.vector.tensor_copy(vaug[t16][:, :, 0:64], ps[:])

        # ---------------- attention for one (i-tile, head) ----------------
        def attn(it, h):
            i0 = it * IW
            njc = i0 // 128 + IW // 128
            npair = njc // 2
            p0 = pj0(it, h)
            yacc = ps_y.tile([128, IW], f32, tag="yacc", name="yacc")
            pend = None

            def emit_pv(p, c0a, c0b, Wa, Wb, pj):
                nc.tensor.matmul(
                    yacc[:, c0a:IW], vaug[2 * pj][:, h, :], p[:, 0:Wa],
                    start=(pj == p0), stop=False)
                nc.tensor.matmul(
                    yacc[:, c0b:IW], vaug[2 * pj + 1][:, h, :], p[:, Wa:Wa + Wb],
                    start=False, stop=(pj == npair - 1))

            for pj in range(p0, npair):
                j0a = (2 * pj) * 128
                j0b = j0a + 128
                c0a = max(0, j0a - i0)
                c0b = max(0, j0b - i0)
                Wa = IW - c0a
                Wb = IW - c0b
                st2 = ps_st.tile([128, 2 * IW], f32, tag="st", name="st")
                nc.tensor.matmul(
                    st2[:, 0:Wa],
                    kaug[h][0:66, j0a:j0a + 128],
                    qaug[h][0:66, i0 + c0a:i0 + IW],
                    start=True, stop=True)
                nc.tensor.matmul(
                    st2[:, Wa:Wa + Wb],
                    kaug[h][0:66, j0b:j0b + 128],
                    qaug[h][0:66, i0 + c0b:i0 + IW],
                    start=True, stop=True)
                if pend is not None:
                    emit_pv(*pend)
                    pend = None
                p = work.tile([128, 2 * IW], fr, tag="p", name="p")
                nc.scalar.activation(p[:, 0:Wa + Wb], st2[:, 0:Wa + Wb], Exp)
                if j0a >= i0:
                    nc.vector.tensor_tensor(p[:, 0:128], p[:, 0:128], umin[:], Min)
                    nc.vector.tensor_tensor(
                        p[:, Wa:Wa + 128], p[:, Wa:Wa + 128], umin[:], Min)
                pend = (p, c0a, c0b, Wa, Wb, pj)
            emit_pv(*pend)
            # denominator rows 64:128 of yacc (broadcast by vaug ones cols)
            den = rpool.tile([64, IW], f32, tag="den", name="den")
            nc.vector.tensor_copy(den[:], yacc[64:128, :])
            rec = rpool.tile([64, IW], f32, tag="rec", name="rec")
            nc.vector.reciprocal_approx_fast(rec[:], den[:])
            nc.vector.tensor_mul(
                yT[h // 2][(h % 2) * 64:(h % 2) * 64 + 64, i0:i0 + IW],
                yacc[0:64, :], rec[:])

        def outproj(it):
            for tp in range(2 * it, 2 * it + 2):
                ot = work.tile([128, 2, C], fr, tag="ot", name="ot")
                for half in range(2):
                    t16 = 2 * tp + half
                    for e2 in range(2):
                        ps = ps_o.tile([128, 512], f32, tag="o", name="ops")
                        for kk in range(2):
                            nc.tensor.matmul(
                                ps[:],
                                yT[kk][:, t16 * 128:(t16 + 1) * 128],
                                wp_t[:, kk, e2 * 512:(e2 + 1) * 512],
                                start=(kk == 0), stop=(kk == 1))
                        nc.vector.tensor_copy(
                            ot[:, half, e2 * 512:(e2 + 1) * 512], ps[:])
                nc.sync.dma_start(out_d[:, 2 * tp:2 * tp + 2, :], ot[:])

        # ---------------- program order ----------------
        for it in range(4):
            qk_tile(0, it)
            qk_tile(2, it)
            for t16 in range(4 * it, 4 * it + 4):
                v_tile(t16)
            if it > 0:
                outproj(it - 1)
            attn(it, 0)
            attn(it, 1)
            qk_tile(1, it)
            qk_tile(3, it)
            attn(it, 2)
            attn(it, 3)
        outproj(3)

    nc.compile()
    return nc


def _get_nc():
    if "nc" not in _CACHE:
        _CACHE["nc"] = _build_nc()
    return _CACHE["nc"]


def _shard_inputs(x, W_attn, b_attn, W_proj, b_proj):
    f16 = np.float16
    slopes = (1.0 / np.power(2.0, np.arange(1, H + 1))).astype(np.float32)
    iota = np.arange(T, dtype=np.float32)
    ak = np.stack([np.ones(T, np.float32), iota]).astype(f16)      # [2, T]
    pp, ff = np.meshgrid(np.arange(128), np.arange(128), indexing="ij")
    umin = np.where(pp <= ff, 30000.0, 0.0).astype(f16)
    # x transposed, quarter-major packed: [128, 4, 8, 512]
    xqs = []
    for b in range(x.shape[0]):
        xT = np.ascontiguousarray(x[b].T).astype(f16)              # [C, T]
        xqs.append(np.ascontiguousarray(
            xT.reshape(8, 128, 4, IW).transpose(1, 2, 0, 3)))

    in_maps = []
    for core in range(8):
        b, g = core // 4, core % 4
        heads = [g, g + 4, g + 8, g + 12]                          # slot 0 banded
        hcols = np.concatenate([np.arange(h * D, (h + 1) * D) for h in heads])
        q_cols = W_attn[:, 0:C][:, hcols] * 0.125
        k_cols = W_attn[:, C:2 * C][:, hcols]
        v_cols = W_attn[:, 2 * C:3 * C][:, hcols]
        wqk_l = np.concatenate([q_cols, k_cols], axis=1)           # [C, 512]
        wqk_p = np.ascontiguousarray(
            wqk_l.reshape(8, 128, 4, 128).transpose(1, 2, 0, 3)).astype(f16)
        bq_l = (b_attn[0:C][hcols] * 0.125).astype(np.float32)     # [256]
        bq_p = np.ascontiguousarray(bq_l.reshape(2, 128).T)        # [128, 2]
        wv_p = np.ascontiguousarray(
            v_cols.reshape(8, 128, HD).transpose(1, 0, 2)).astype(f16)
        wp_l = W_proj[hcols, :]                                    # [256, C]
        wp_p = np.ascontiguousarray(
            wp_l.reshape(2, 128, C).transpose(1, 0, 2)).astype(f16)
        aq = np.zeros((2 * HL, T), np.float32)
        for hh in range(HL):
            s = slopes[heads[hh]]
            aq[2 * hh, :] = -s * iota
            aq[2 * hh + 1, :] = s
        in_maps.append({
            "xq": xqs[b], "wqk": wqk_p, "bq": bq_p,
            "wv": wv_p, "wp": wp_p,
            "aq": aq.astype(f16), "ak": ak, "umin": umin,
        })
    return in_maps


def kernel(x, W_attn, b_attn, W_proj, b_proj, _trace=False, _tmpdir=None):
    from concourse.bass_utils import run_bass_kernel_spmd

    x = np.asarray(x, dtype=np.float32)
    W_attn = np.asarray(W_attn, dtype=np.float32)
    b_attn = np.asarray(b_attn, dtype=np.float32)
    W_proj = np.asarray(W_proj, dtype=np.float32)
    b_proj = np.asarray(b_proj, dtype=np.float32)

    nc = _get_nc()
    in_maps = _shard_inputs(x, W_attn, b_attn, W_proj, b_proj)
    res = run_bass_kernel_spmd(
        nc, in_maps, core_ids=list(range(8)), trace=_trace, tmpdir=_tmpdir)
    # v-bias commutes through softmax: fold bv @ W_proj into the output bias.
    b_eff = b_proj + b_attn[2 * C:3 * C] @ W_proj
    out = np.empty((x.shape[0], T, C), np.float32)
    for b in range(x.shape[0]):
        acc = None
        for g in range(4):
            o = res.results[4 * b + g]["out"].astype(np.float32)   # [128,16,C]
            o = o.transpose(1, 0, 2).reshape(T, C)
            acc = o if acc is None else acc + o
        out[b] = acc + b_eff
    if _trace:
        kernel.last_exec_time_ns = res.exec_time_ns
    return out


# revision 9
# speedup vs baseline: 1.2191x; 1.0343x over previous
"""AlibiCausalSelfAttention on 8 Trainium2 NeuronCores — v3.

Sharding: data-parallel over batch (B=2) x head-parallel over head groups
(16 heads -> 4 groups of 4, strided: group g = {g, g+4, g+8, g+12}).
Core c handles batch c//4, head group c%4. Each core computes a partial
projection output fp16 (W_proj row-sharded); the host sums the 4 partials
per batch in fp32 and adds an effective bias (b_proj + bv @ W_proj — the
v-bias commutes through softmax exactly).

v3 structural changes vs v2 (175us):
  - All bulk DMAs use host-prepacked per-partition-contiguous layouts
    (~128 descriptors/MB instead of ~1000), cutting DGE trigger time and
    letting the input stream start right after the ~7us framework preamble.
  - Bias matmuls eliminated: v-bias folded into the host-side output bias
    (exact); k-bias dropped (q.bk + bq.bk are per-row constants — exact
    softmax invariance); q-bias applied during the PSUM->SBUF evacuation
    via tensor_scalar_add (free).
  - Warmup junk matmuls read an uninitialized SBUF tile (no memset dep),
    so they issue as soon as the PE queue preamble ends and keep the HAM
    clock gate warm through the input-DMA window.
  - Banded attention for the steepest head of each group (global heads
    0-3, local slot 0): only j-pairs within 512 tokens of the diagonal are
    computed (dropped softmax terms < e^-18 relative — far below fp16
    resolution). Identical program on all cores; ~9% less QK/PV/exp work.
  - v-evacuation merged to one strided copy per t-chunk; softmax
    normalization reads the denominator rows straight out of PSUM
    (reciprocal_approx_fast), dropping the staging copy.
  - vaug ones-columns memset on the idle GPSIMD engine.
"""

import sys

if "/opt/trn_rl_repo" not in sys.path:
    sys.path.insert(0, "/opt/trn_rl_repo")

import numpy as np

T = 2048
C = 1024
H = 16
D = 64
HL = 4          # heads per core
HD = HL * D     # 256 local head dims
IW = 512        # i-tile width
NT16 = T // 128
NW = 10         # warmup junk matmuls

# local-slot bands (tokens below diagonal, multiple of 256; None = full).
# Slot 0 carries global heads 0..3 (slopes 1/2..1/16): need >= 18/slope =
# 288 tokens; 512 is safely generous. Slots 1-3 carry heads 4-15 (full).
BANDS = [512, None, None, None]

_CACHE = {}


def _build_nc():
    import concourse.mybir as mybir
    import concourse.tile as tile
    from concourse import bacc
    from contextlib import ExitStack

    f32 = mybir.dt.float32
    fr = mybir.dt.float16
    Exp = mybir.ActivationFunctionType.Exp
    Copy = mybir.ActivationFunctionType.Copy
    Min = mybir.AluOpType.min

    def pj0(it, h):
        b = BANDS[h]
        if b is None:
            return 0
        return max(0, (it * IW - b) // 256)

    nc = bacc.Bacc("TRN2", target_bir_lowering=False, debug=False, num_devices=8)

    xq_d = nc.dram_tensor("xq", [128, 4, 8, IW], fr, kind="ExternalInput").ap()
    wqk02_d = nc.dram_tensor("wqk02", [128, 2, 8, 128], fr, kind="ExternalInput").ap()
    wqk13_d = nc.dram_tensor("wqk13", [128, 2, 8, 128], fr, kind="ExternalInput").ap()
    bq_d = nc.dram_tensor("bq", [128, 2], f32, kind="ExternalInput").ap()
    wv_d = nc.dram_tensor("wv", [128, 8, HD], fr, kind="ExternalInput").ap()
    wp_d = nc.dram_tensor("wp", [128, 2, C], fr, kind="ExternalInput").ap()
    aq_d = nc.dram_tensor("aq", [2 * HL, T], fr, kind="ExternalInput").ap()
    ak_d = nc.dram_tensor("ak", [2, T], fr, kind="ExternalInput").ap()
    umin_d = nc.dram_tensor("umin", [128, 128], fr, kind="ExternalInput").ap()
    out_d = nc.dram_tensor("out", [128, NT16, C], fr, kind="ExternalOutput").ap()

    with tile.TileContext(nc) as tc, ExitStack() as ctx:
        pers = ctx.enter_context(tc.tile_pool(name="pers", bufs=1))
        work = ctx.enter_context(tc.tile_pool(name="work", bufs=3))
        rpool = ctx.enter_context(tc.tile_pool(name="rpool", bufs=2))
        ps_st = ctx.enter_context(tc.tile_pool(name="ps_st", bufs=2, space="PSUM"))
        ps_y = ctx.enter_context(tc.tile_pool(name="ps_y", bufs=2, space="PSUM"))
        ps_o = ctx.enter_context(tc.tile_pool(name="ps_o", bufs=2, space="PSUM"))

        # ---- persistent tiles ----
        qaug = [pers.tile([128, T], fr, tag=f"qaug{h}", name=f"qaug{h}") for h in range(HL)]
        kaug = [pers.tile([128, T], fr, tag=f"kaug{h}", name=f"kaug{h}") for h in range(HL)]
        vaug = [pers.tile([128, HL, 128], fr, tag=f"vaug{t}", name=f"vaug{t}")
                for t in range(NT16)]
        yT = [pers.tile([128, T], fr, tag=f"yT{i}", name=f"yT{i}") for i in range(HL // 2)]
        xs_q = [pers.tile([128, 8, IW], fr, tag=f"xs{q}", name=f"xs{q}")
                for q in range(4)]
        wqk02 = pers.tile([128, 2, 8, 128], fr, tag="wqk02")
        wqk13 = pers.tile([128, 2, 8, 128], fr, tag="wqk13")
        wv_t = pers.tile([128, 8, HD], fr, tag="wvt")
        wp_t = pers.tile([128, 2, C], fr, tag="wpt")
        bq_t = pers.tile([128, 2], f32, tag="bq")
        umin = pers.tile([128, 128], fr, tag="umin")
        junk = pers.tile([128, IW], fr, tag="junk")

        # ---- warmup junk matmuls: minimal deps (one gpsimd memset), keep the
        # PE/HAM warm while inputs stream in.
        nc.gpsimd.memset(junk[:], 0.0)
        for w in range(NW):
            psd = ps_st.tile([128, IW], f32, tag="st", name="psd")
            nc.tensor.matmul(psd[:], junk[:, 0:128], junk[:], start=True, stop=True)

        # ---- DMAs: per-partition-contiguous both sides; ordered so the
        # minimal prefix for early compute lands first.
        nc.sync.dma_start(wqk02[:], wqk02_d[:])
        nc.sync.dma_start(xs_q[0][:, 0:4, :], xq_d[:, 0, 0:4, :])
        nc.sync.dma_start(xs_q[0][:, 4:8, :], xq_d[:, 0, 4:8, :])
        nc.sync.dma_start(wv_t[:], wv_d[:])
        nc.sync.dma_start(wqk13[:], wqk13_d[:])
        nc.sync.dma_start(xs_q[1][:], xq_d[:, 1, :, :])
        nc.sync.dma_start(wp_t[:], wp_d[:])
        nc.sync.dma_start(xs_q[2][:], xq_d[:, 2, :, :])
        nc.sync.dma_start(xs_q[3][:], xq_d[:, 3, :, :])
        # small constants on the scalar HWDGE queue
        nc.scalar.dma_start(bq_t[:], bq_d[:])
        nc.scalar.dma_start(umin[:], umin_d[:])
        for h in range(HL):
            nc.scalar.dma_start(qaug[h][64:66, :], aq_d[2 * h:2 * h + 2, :])
            nc.scalar.dma_start(kaug[h][64:66, :], ak_d[:, :])
        # ones columns of vaug on the idle GPSIMD engine
        for t in range(NT16):
            nc.gpsimd.memset(vaug[t][:, :, 64:128], 1.0)

        # ---------------- phase 1 helpers ----------------
        def qk_tile(cc, it):
            ps = ps_o.tile([128, IW], f32, tag="o", name="qkps")
            wt = wqk02 if cc % 2 == 0 else wqk13
            for k in range(8):
                nc.tensor.matmul(
                    ps[:], wt[:, cc // 2, k, :], xs_q[it][:, k, :],
                    start=(k == 0), stop=(k == 7))
            for half in range(2):
                h = (cc % 2) * 2 + half
                src = ps[half * 64:(half + 1) * 64, :]
                if cc < 2:
                    nc.vector.tensor_scalar_add(
                        qaug[h][0:64, it * IW:(it + 1) * IW], src,
                        bq_t[half * 64:(half + 1) * 64, cc:cc + 1])
                else:
                    nc.scalar.activation(
                        kaug[h][0:64, it * IW:(it + 1) * IW], src, Copy)

        def v_tile(t16):
            ps = ps_o.tile([128, HL, 64], f32, tag="o", name="vps")
            qq, sub = t16 // 4, (t16 % 4) * 128
            for k in range(8):
                nc.tensor.matmul(
                    ps[:], xs_q[qq][:, k, sub:sub + 128], wv_t[:, k, :],
                    start=(k == 0), stop=(k == 7))
            nc.scalar.activation(vaug[t16][:, :, 0:64], ps[:], Copy)

        # ---------------- attention for one (i-tile, head) ----------------
        def attn(it, h):
            i0 = it * IW
            njc = i0 // 128 + IW // 128
            npair = njc // 2
            p0 = pj0(it, h)
            yacc = ps_y.tile([128, IW], f32, tag="yacc", name="yacc")
            pend = None

            def emit_pv(p, c0a, c0b, Wa, Wb, pj):
                nc.tensor.matmul(
                    yacc[:, c0a:IW], vaug[2 * pj][:, h, :], p[:, 0:Wa],
                    start=(pj == p0), stop=False)
                nc.tensor.matmul(
                    yacc[:, c0b:IW], vaug[2 * pj + 1][:, h, :], p[:, Wa:Wa + Wb],
                    start=False, stop=(pj == npair - 1))

            for pj in range(p0, npair):
                j0a = (2 * pj) * 128
                j0b = j0a + 128
                c0a = max(0, j0a - i0)
                c0b = max(0, j0b - i0)
                Wa = IW - c0a
                Wb = IW - c0b
                st2 = ps_st.tile([128, 2 * IW], f32, tag="st", name="st")
                nc.tensor.matmul(
                    st2[:, 0:Wa],
                    kaug[h][0:66, j0a:j0a + 128],
                    qaug[h][0:66, i0 + c0a:i0 + IW],
                    start=True, stop=True)
                nc.tensor.matmul(
                    st2[:, Wa:Wa + Wb],
                    kaug[h][0:66, j0b:j0b + 128],
                    qaug[h][0:66, i0 + c0b:i0 + IW],
                    start=True, stop=True)
                if pend is not None:
                    emit_pv(*pend)
                    pend = None
                p = work.tile([128, 2 * IW], fr, tag="p", name="p")
                nc.scalar.activation(p[:, 0:Wa + Wb], st2[:, 0:Wa + Wb], Exp)
                if j0a >= i0:
                    nc.vector.tensor_tensor(p[:, 0:128], p[:, 0:128], umin[:], Min)
                    nc.vector.tensor_tensor(
                        p[:, Wa:Wa + 128], p[:, Wa:Wa + 128], umin[:], Min)
                pend = (p, c0a, c0b, Wa, Wb, pj)
            emit_pv(*pend)
            # denominator rows 64:128 of yacc (broadcast by vaug ones cols)
            den = rpool.tile([64, IW], f32, tag="den", name="den")
            nc.vector.tensor_copy(den[:], yacc[64:128, :])
            rec = rpool.tile([64, IW], f32, tag="rec", name="rec")
            nc.vector.reciprocal_approx_fast(rec[:], den[:])
            nc.vector.tensor_mul(
                yT[h // 2][(h % 2) * 64:(h % 2) * 64 + 64, i0:i0 + IW],
                yacc[0:64, :], rec[:])

        def outproj(it):
            for tp in range(2 * it, 2 * it + 2):
                ot = work.tile([128, 2, C], fr, tag="ot", name="ot")
                for half in range(2):
                    t16 = 2 * tp + half
                    for e2 in range(2):
                        ps = ps_o.tile([128, 512], f32, tag="o", name="ops")
                        for kk in range(2):
                            nc.tensor.matmul(
                                ps[:],
                                yT[kk][:, t16 * 128:(t16 + 1) * 128],
                                wp_t[:, kk, e2 * 512:(e2 + 1) * 512],
                                start=(kk == 0), stop=(kk == 1))
                        nc.vector.tensor_copy(
                            ot[:, half, e2 * 512:(e2 + 1) * 512], ps[:])
                nc.sync.dma_start(out_d[:, 2 * tp:2 * tp + 2, :], ot[:])

        # ---------------- program order ----------------
        for it in range(4):
            qk_tile(0, it)
            qk_tile(2, it)
            for t16 in range(4 * it, 4 * it + 4):
                v_tile(t16)
            if it > 0:
                outproj(it - 1)
            attn(it, 0)
            attn(it, 1)
            qk_tile(1, it)
            qk_tile(3, it)
            attn(it, 2)
            attn(it, 3)
        outproj(3)

    nc.compile()
    return nc


def _get_nc():
    if "nc" not in _CACHE:
        _CACHE["nc"] = _build_nc()
    return _CACHE["nc"]


def _shard_inputs(x, W_attn, b_attn, W_proj, b_proj):
    f16 = np.float16
    slopes = (1.0 / np.power(2.0, np.arange(1, H + 1))).astype(np.float32)
    iota = np.arange(T, dtype=np.float32)
    ak = np.stack([np.ones(T, np.float32), iota]).astype(f16)      # [2, T]
    pp, ff = np.meshgrid(np.arange(128), np.arange(128), indexing="ij")
    umin = np.where(pp <= ff, 30000.0, 0.0).astype(f16)
    # x transposed, quarter-major packed: [128, 4, 8, 512]
    xqs = []
    for b in range(x.shape[0]):
        xT = np.ascontiguousarray(x[b].T).astype(f16)              # [C, T]
        xqs.append(np.ascontiguousarray(
            xT.reshape(8, 128, 4, IW).transpose(1, 2, 0, 3)))

    in_maps = []
    for core in range(8):
        b, g = core // 4, core % 4
        heads = [g, g + 4, g + 8, g + 12]                          # slot 0 banded
        hcols = np.concatenate([np.arange(h * D, (h + 1) * D) for h in heads])
        q_cols = W_attn[:, 0:C][:, hcols] * 0.125
        k_cols = W_attn[:, C:2 * C][:, hcols]
        v_cols = W_attn[:, 2 * C:3 * C][:, hcols]
        wqk_l = np.concatenate([q_cols, k_cols], axis=1)           # [C, 512]
        wqk_a = wqk_l.reshape(8, 128, 4, 128)                      # (k,p,cc,c)
        wqk02_p = np.ascontiguousarray(
            wqk_a[:, :, [0, 2], :].transpose(1, 2, 0, 3)).astype(f16)
        wqk13_p = np.ascontiguousarray(
            wqk_a[:, :, [1, 3], :].transpose(1, 2, 0, 3)).astype(f16)
        bq_l = (b_attn[0:C][hcols] * 0.125).astype(np.float32)     # [256]
        bq_p = np.ascontiguousarray(bq_l.reshape(2, 128).T)        # [128, 2]
        wv_p = np.ascontiguousarray(
            v_cols.reshape(8, 128, HD).transpose(1, 0, 2)).astype(f16)
        wp_l = W_proj[hcols, :]                                    # [256, C]
        wp_p = np.ascontiguousarray(
            wp_l.reshape(2, 128, C).transpose(1, 0, 2)).astype(f16)
        aq = np.zeros((2 * HL, T), np.float32)
        for hh in range(HL):
            s = slopes[heads[hh]]
            aq[2 * hh, :] = -s * iota
            aq[2 * hh + 1, :] = s
        in_maps.append({
            "xq": xqs[b], "wqk02": wqk02_p, "wqk13": wqk13_p, "bq": bq_p,
            "wv": wv_p, "wp": wp_p,
            "aq": aq.astype(f16), "ak": ak, "umin": umin,
        })
    return in_maps


def kernel(x, W_attn, b_attn, W_proj, b_proj, _trace=False, _tmpdir=None):
    from concourse.bass_utils import run_bass_kernel_spmd

    x = np.asarray(x, dtype=np.float32)
    W_attn = np.asarray(W_attn, dtype=np.float32)
    b_attn = np.asarray(b_attn, dtype=np.float32)
    W_proj = np.asarray(W_proj, dtype=np.float32)
    b_proj = np.asarray(b_proj, dtype=np.float32)

    nc = _get_nc()
    in_maps = _shard_inputs(x, W_attn, b_attn, W_proj, b_proj)
    res = run_bass_kernel_spmd(
        nc, in_maps, core_ids=list(range(8)), trace=_trace, tmpdir=_tmpdir)
    # v-bias commutes through softmax: fold bv @ W_proj into the output bias.
    b_eff = b_proj + b_attn[2 * C:3 * C] @ W_proj
    out = np.empty((x.shape[0], T, C), np.float32)
    for b in range(x.shape[0]):
        acc = None
        for g in range(4):
            o = res.results[4 * b + g]["out"].astype(np.float32)   # [128,16,C]
            o = o.transpose(1, 0, 2).reshape(T, C)
            acc = o if acc is None else acc + o
        out[b] = acc + b_eff
    if _trace:
        kernel.last_exec_time_ns = res.exec_time_ns
    return out


# revision 10
# speedup vs baseline: 1.3078x; 1.0728x over previous
"""AlibiCausalSelfAttention on 8 Trainium2 NeuronCores — v3.

Sharding: data-parallel over batch (B=2) x head-parallel over head groups
(16 heads -> 4 groups of 4, strided: group g = {g, g+4, g+8, g+12}).
Core c handles batch c//4, head group c%4. Each core computes a partial
projection output fp16 (W_proj row-sharded); the host sums the 4 partials
per batch in fp32 and adds an effective bias (b_proj + bv @ W_proj — the
v-bias commutes through softmax exactly).

v3 structural changes vs v2 (175us):
  - All bulk DMAs use host-prepacked per-partition-contiguous layouts
    (~128 descriptors/MB instead of ~1000), cutting DGE trigger time and
    letting the input stream start right after the ~7us framework preamble.
  - Bias matmuls eliminated: v-bias folded into the host-side output bias
    (exact); k-bias dropped (q.bk + bq.bk are per-row constants — exact
    softmax invariance); q-bias applied during the PSUM->SBUF evacuation
    via tensor_scalar_add (free).
  - Warmup junk matmuls read an uninitialized SBUF tile (no memset dep),
    so they issue as soon as the PE queue preamble ends and keep the HAM
    clock gate warm through the input-DMA window.
  - Banded attention for the steepest head of each group (global heads
    0-3, local slot 0): only j-pairs within 512 tokens of the diagonal are
    computed (dropped softmax terms < e^-18 relative — far below fp16
    resolution). Identical program on all cores; ~9% less QK/PV/exp work.
  - v-evacuation merged to one strided copy per t-chunk; softmax
    normalization reads the denominator rows straight out of PSUM
    (reciprocal_approx_fast), dropping the staging copy.
  - vaug ones-columns memset on the idle GPSIMD engine.
"""

import sys

if "/opt/trn_rl_repo" not in sys.path:
    sys.path.insert(0, "/opt/trn_rl_repo")

import numpy as np

T = 2048
C = 1024
H = 16
D = 64
HL = 4          # heads per core
HD = HL * D     # 256 local head dims
IW = 512        # i-tile width
NT16 = T // 128
NW = 10         # warmup junk matmuls

# local-slot bands (tokens below diagonal, multiple of 256; None = full).
# Slot 0 carries global heads 0..3 (slopes 1/2..1/16): need >= 18/slope =
# 288 tokens; 512 is safely generous. Slots 1-3 carry heads 4-15 (full).
BANDS = [512, None, None, None]

_CACHE = {}


def _build_nc():
    import concourse.mybir as mybir
    import concourse.tile as tile
    from concourse import bacc
    from contextlib import ExitStack

    f32 = mybir.dt.float32
    fr = mybir.dt.float16
    f8 = mybir.dt.float8e4
    DR = mybir.MatmulPerfMode.DoubleRow
    Mult = mybir.AluOpType.mult
    Add = mybir.AluOpType.add
    Exp = mybir.ActivationFunctionType.Exp
    Copy = mybir.ActivationFunctionType.Copy
    Min = mybir.AluOpType.min

    def pj0(it, h):
        b = BANDS[h]
        if b is None:
            return 0
        return max(0, (it * IW - b) // 256)

    nc = bacc.Bacc("TRN2", target_bir_lowering=False, debug=False, num_devices=8)

    xq_d = nc.dram_tensor("xq", [128, 4, 8, IW], fr, kind="ExternalInput").ap()
    x8_d = nc.dram_tensor("x8", [128, 4, 4, 2, IW], f8, kind="ExternalInput").ap()
    wqk8_d = nc.dram_tensor("wqk8", [128, 4, 4, 2, 128], f8, kind="ExternalInput").ap()
    bq_d = nc.dram_tensor("bq", [128, 2], f32, kind="ExternalInput").ap()
    wv_d = nc.dram_tensor("wv", [128, 8, HD], fr, kind="ExternalInput").ap()
    wp_d = nc.dram_tensor("wp", [128, 2, C], fr, kind="ExternalInput").ap()
    aq_d = nc.dram_tensor("aq", [2 * HL, T], fr, kind="ExternalInput").ap()
    ak_d = nc.dram_tensor("ak", [2, T], fr, kind="ExternalInput").ap()
    umin_d = nc.dram_tensor("umin", [128, 128], fr, kind="ExternalInput").ap()
    out_d = nc.dram_tensor("out", [128, NT16, C], fr, kind="ExternalOutput").ap()

    with tile.TileContext(nc) as tc, ExitStack() as ctx:
        pers = ctx.enter_context(tc.tile_pool(name="pers", bufs=1))
        work = ctx.enter_context(tc.tile_pool(name="work", bufs=3))
        rpool = ctx.enter_context(tc.tile_pool(name="rpool", bufs=2))
        ps_st = ctx.enter_context(tc.tile_pool(name="ps_st", bufs=2, space="PSUM"))
        ps_y = ctx.enter_context(tc.tile_pool(name="ps_y", bufs=2, space="PSUM"))
        ps_o = ctx.enter_context(tc.tile_pool(name="ps_o", bufs=2, space="PSUM"))

        # ---- persistent tiles ----
        qaug = [pers.tile([128, T], fr, tag=f"qaug{h}", name=f"qaug{h}") for h in range(HL)]
        kaug = [pers.tile([128, T], fr, tag=f"kaug{h}", name=f"kaug{h}") for h in range(HL)]
        vaug = [pers.tile([128, HL, 128], fr, tag=f"vaug{t}", name=f"vaug{t}")
                for t in range(NT16)]
        yT = [pers.tile([128, T], fr, tag=f"yT{i}", name=f"yT{i}") for i in range(HL // 2)]
        xs_q = [pers.tile([128, 8, IW], fr, tag=f"xs{q}", name=f"xs{q}")
                for q in range(4)]
        x8_q = [pers.tile([128, 4, 2, IW], f8, tag=f"x8{q}", name=f"x8{q}")
                for q in range(4)]
        wqk8_t = pers.tile([128, 4, 4, 2, 128], f8, tag="wqk8")
        wv_t = pers.tile([128, 8, HD], fr, tag="wvt")
        wp_t = pers.tile([128, 2, C], fr, tag="wpt")
        bq_t = pers.tile([128, 2], f32, tag="bq")
        umin = pers.tile([128, 128], fr, tag="umin")
        junk = pers.tile([128, IW], fr, tag="junk")

        # ---- warmup junk matmuls: minimal deps (one gpsimd memset), keep the
        # PE/HAM warm while inputs stream in.
        nc.gpsimd.memset(junk[:], 0.0)
        for w in range(NW):
            psd = ps_st.tile([128, IW], f32, tag="st", name="psd")
            nc.tensor.matmul(psd[:], junk[:, 0:128], junk[:], start=True, stop=True)

        # ---- DMAs: per-partition-contiguous both sides; ordered so the
        # minimal prefix for early compute lands first.
        nc.sync.dma_start(wqk8_t[:], wqk8_d[:])
        nc.sync.dma_start(x8_q[0][:], x8_d[:, 0, :, :, :])
        nc.sync.dma_start(xs_q[0][:, 0:4, :], xq_d[:, 0, 0:4, :])
        nc.sync.dma_start(xs_q[0][:, 4:8, :], xq_d[:, 0, 4:8, :])
        nc.sync.dma_start(wv_t[:], wv_d[:])
        nc.sync.dma_start(x8_q[1][:], x8_d[:, 1, :, :, :])
        nc.sync.dma_start(xs_q[1][:], xq_d[:, 1, :, :])
        nc.sync.dma_start(wp_t[:], wp_d[:])
        nc.sync.dma_start(x8_q[2][:], x8_d[:, 2, :, :, :])
        nc.sync.dma_start(xs_q[2][:], xq_d[:, 2, :, :])
        nc.sync.dma_start(x8_q[3][:], x8_d[:, 3, :, :, :])
        nc.sync.dma_start(xs_q[3][:], xq_d[:, 3, :, :])
        # small constants on the scalar HWDGE queue
        nc.scalar.dma_start(bq_t[:], bq_d[:])
        nc.scalar.dma_start(umin[:], umin_d[:])
        for h in range(HL):
            nc.scalar.dma_start(qaug[h][64:66, :], aq_d[2 * h:2 * h + 2, :])
            nc.scalar.dma_start(kaug[h][64:66, :], ak_d[:, :])
        # ones columns of vaug on the idle GPSIMD engine
        for t in range(NT16):
            nc.gpsimd.memset(vaug[t][:, :, 64:128], 1.0)

        # ---------------- phase 1 helpers ----------------
        def qk_tile(cc, it):
            ps = ps_o.tile([128, IW], f32, tag="o", name="qkps")
            for kp in range(4):
                nc.tensor.matmul(
                    ps[:], wqk8_t[:, cc, kp, :, :], x8_q[it][:, kp, :, :],
                    start=(kp == 0), stop=(kp == 3), perf_mode=DR)
            for half in range(2):
                h = (cc % 2) * 2 + half
                src = ps[half * 64:(half + 1) * 64, :]
                if cc < 2:
                    nc.vector.tensor_scalar(
                        qaug[h][0:64, it * IW:(it + 1) * IW], src,
                        1.0 / 256.0, bq_t[half * 64:(half + 1) * 64, cc:cc + 1],
                        Mult, Add)
                else:
                    nc.scalar.activation(
                        kaug[h][0:64, it * IW:(it + 1) * IW], src, Copy,
                        scale=1.0 / 32.0)

        def v_tile(t16):
            ps = ps_o.tile([128, HL, 64], f32, tag="o", name="vps")
            qq, sub = t16 // 4, (t16 % 4) * 128
            for k in range(8):
                nc.tensor.matmul(
                    ps[:], xs_q[qq][:, k, sub:sub + 128], wv_t[:, k, :],
                    start=(k == 0), stop=(k == 7))
            nc.scalar.activation(vaug[t16][:, :, 0:64], ps[:], Copy)

        # ---------------- attention for one (i-tile, head) ----------------
        def attn(it, h):
            i0 = it * IW
            njc = i0 // 128 + IW // 128
            npair = njc // 2
            p0 = pj0(it, h)
            yacc = ps_y.tile([128, IW], f32, tag="yacc", name="yacc")
            pend = None

            def emit_pv(p, c0a, c0b, Wa, Wb, pj):
                nc.tensor.matmul(
                    yacc[:, c0a:IW], vaug[2 * pj][:, h, :], p[:, 0:Wa],
                    start=(pj == p0), stop=False)
                nc.tensor.matmul(
                    yacc[:, c0b:IW], vaug[2 * pj + 1][:, h, :], p[:, Wa:Wa + Wb],
                    start=False, stop=(pj == npair - 1))

            for pj in range(p0, npair):
                j0a = (2 * pj) * 128
                j0b = j0a + 128
                c0a = max(0, j0a - i0)
                c0b = max(0, j0b - i0)
                Wa = IW - c0a
                Wb = IW - c0b
                st2 = ps_st.tile([128, 2 * IW], f32, tag="st", name="st")
                nc.tensor.matmul(
                    st2[:, 0:Wa],
                    kaug[h][0:66, j0a:j0a + 128],
                    qaug[h][0:66, i0 + c0a:i0 + IW],
                    start=True, stop=True)
                nc.tensor.matmul(
                    st2[:, Wa:Wa + Wb],
                    kaug[h][0:66, j0b:j0b + 128],
                    qaug[h][0:66, i0 + c0b:i0 + IW],
                    start=True, stop=True)
                if pend is not None:
                    emit_pv(*pend)
                    pend = None
                p = work.tile([128, 2 * IW], fr, tag="p", name="p")
                nc.scalar.activation(p[:, 0:Wa + Wb], st2[:, 0:Wa + Wb], Exp)
                if j0a >= i0:
                    nc.vector.tensor_tensor(p[:, 0:128], p[:, 0:128], umin[:], Min)
                    nc.vector.tensor_tensor(
                        p[:, Wa:Wa + 128], p[:, Wa:Wa + 128], umin[:], Min)
                pend = (p, c0a, c0b, Wa, Wb, pj)
            emit_pv(*pend)
            # denominator rows 64:128 of yacc (broadcast by vaug ones cols)
            den = rpool.tile([64, IW], f32, tag="den", name="den")
            nc.vector.tensor_copy(den[:], yacc[64:128, :])
            rec = rpool.tile([64, IW], f32, tag="rec", name="rec")
            nc.vector.reciprocal_approx_fast(rec[:], den[:])
            nc.vector.tensor_mul(
                yT[h // 2][(h % 2) * 64:(h % 2) * 64 + 64, i0:i0 + IW],
                yacc[0:64, :], rec[:])

        def outproj_tp(tp):
                ot = work.tile([128, 2, C], fr, tag="ot", name="ot")
                for half in range(2):
                    t16 = 2 * tp + half
                    for e2 in range(2):
                        ps = ps_o.tile([128, 512], f32, tag="o", name="ops")
                        for kk in range(2):
                            nc.tensor.matmul(
                                ps[:],
                                yT[kk][:, t16 * 128:(t16 + 1) * 128],
                                wp_t[:, kk, e2 * 512:(e2 + 1) * 512],
                                start=(kk == 0), stop=(kk == 1))
                        nc.vector.tensor_copy(
                            ot[:, half, e2 * 512:(e2 + 1) * 512], ps[:])
                nc.sync.dma_start(out_d[:, 2 * tp:2 * tp + 2, :], ot[:])


        # ---------------- program order ----------------
        for it in range(4):
            qk_tile(0, it)
            qk_tile(2, it)
            for t16 in range(4 * it, 4 * it + 4):
                v_tile(t16)
            if it > 0:
                outproj_tp(2 * (it - 1))
            attn(it, 0)
            attn(it, 1)
            qk_tile(1, it)
            qk_tile(3, it)
            if it > 0:
                outproj_tp(2 * (it - 1) + 1)
            attn(it, 2)
            attn(it, 3)
        outproj_tp(6)
        outproj_tp(7)

    nc.compile()
    return nc


def _get_nc():
    if "nc" not in _CACHE:
        _CACHE["nc"] = _build_nc()
    return _CACHE["nc"]


def _shard_inputs(x, W_attn, b_attn, W_proj, b_proj):
    import ml_dtypes
    f16 = np.float16
    e4 = ml_dtypes.float8_e4m3
    slopes = (1.0 / np.power(2.0, np.arange(1, H + 1))).astype(np.float32)
    iota = np.arange(T, dtype=np.float32)
    ak = np.stack([np.ones(T, np.float32), iota]).astype(f16)      # [2, T]
    pp, ff = np.meshgrid(np.arange(128), np.arange(128), indexing="ij")
    umin = np.where(pp <= ff, 30000.0, 0.0).astype(f16)
    # x transposed, quarter-major packed: [128, 4, 8, 512]
    xqs, x8s = [], []
    for b in range(x.shape[0]):
        xT = np.ascontiguousarray(x[b].T)                          # [C, T] f32
        xqs.append(np.ascontiguousarray(
            xT.reshape(8, 128, 4, IW).transpose(1, 2, 0, 3)).astype(f16))
        x8s.append(np.ascontiguousarray(
            np.clip(xT, -240, 240).reshape(4, 2, 128, 4, IW)
            .transpose(2, 3, 0, 1, 4)).astype(e4))

    in_maps = []
    for core in range(8):
        b, g = core // 4, core % 4
        heads = [g, g + 4, g + 8, g + 12]                          # slot 0 banded
        hcols = np.concatenate([np.arange(h * D, (h + 1) * D) for h in heads])
        q_cols = W_attn[:, 0:C][:, hcols] * 32.0    # 0.125 sm-scale * 256
        k_cols = W_attn[:, C:2 * C][:, hcols] * 32.0
        v_cols = W_attn[:, 2 * C:3 * C][:, hcols]
        wqk_l = np.concatenate([q_cols, k_cols], axis=1)           # [C, 512]
        wqk8_p = np.ascontiguousarray(
            np.clip(wqk_l, -240, 240).reshape(4, 2, 128, 4, 128)
            .transpose(2, 3, 0, 1, 4)).astype(e4)
        bq_l = (b_attn[0:C][hcols] * 0.125).astype(np.float32)     # [256]
        bq_p = np.ascontiguousarray(bq_l.reshape(2, 128).T)        # [128, 2]
        wv_p = np.ascontiguousarray(
            v_cols.reshape(8, 128, HD).transpose(1, 0, 2)).astype(f16)
        wp_l = W_proj[hcols, :]                                    # [256, C]
        wp_p = np.ascontiguousarray(
            wp_l.reshape(2, 128, C).transpose(1, 0, 2)).astype(f16)
        aq = np.zeros((2 * HL, T), np.float32)
        for hh in range(HL):
            s = slopes[heads[hh]]
            aq[2 * hh, :] = -s * iota
            aq[2 * hh + 1, :] = s
        in_maps.append({
            "xq": xqs[b], "x8": x8s[b], "wqk8": wqk8_p, "bq": bq_p,
            "wv": wv_p, "wp": wp_p,
            "aq": aq.astype(f16), "ak": ak, "umin": umin,
        })
    return in_maps


def kernel(x, W_attn, b_attn, W_proj, b_proj, _trace=False, _tmpdir=None):
    from concourse.bass_utils import run_bass_kernel_spmd

    x = np.asarray(x, dtype=np.float32)
    W_attn = np.asarray(W_attn, dtype=np.float32)
    b_attn = np.asarray(b_attn, dtype=np.float32)
    W_proj = np.asarray(W_proj, dtype=np.float32)
    b_proj = np.asarray(b_proj, dtype=np.float32)

    nc = _get_nc()
    in_maps = _shard_inputs(x, W_attn, b_attn, W_proj, b_proj)
    res = run_bass_kernel_spmd(
        nc, in_maps, core_ids=list(range(8)), trace=_trace, tmpdir=_tmpdir)
    # v-bias commutes through softmax: fold bv @ W_proj into the output bias.
    b_eff = b_proj + b_attn[2 * C:3 * C] @ W_proj
    out = np.empty((x.shape[0], T, C), np.float32)
    for b in range(x.shape[0]):
        acc = None
        for g in range(4):
            o = res.results[4 * b + g]["out"].astype(np.float32)   # [128,16,C]
            o = o.transpose(1, 0, 2).reshape(T, C)
            acc = o if acc is None else acc + o
        out[b] = acc + b_eff
    if _trace:
        kernel.last_exec_time_ns = res.exec_time_ns
    return out
